# revision 1
# baseline (speedup 1.0000x reference)
"""Trainium2 Bass kernel for BoltzmannMoE (top-2 of 8 experts, N=8192, D=1024, H=4096, O=1024).

Strategy (expert-parallel across 8 NeuronCores):
  - Host: gate (softmax -> top-2 -> renormalize) in numpy fp32, gather each
    expert's tokens, run one expert per core, weighted scatter-add on host.
  - Device (per core, SPMD): y_e^T = W2_e^T @ relu(W1_e^T @ xg_e^T + b1_e)
    as two chained f32r matmuls (full PE rate, ~1e-4 matmul precision).
    Tokens processed in passes so hT + xgT stay SBUF-resident; W1/W2 are
    streamed per pass in contiguous 512KB tiles.
"""

import numpy as np

import concourse.bass as bass
import concourse.mybir as mybir
import concourse.tile as tile
from concourse import bacc
from concourse.bass_utils import run_bass_kernel_spmd

P = 128
D, H, O, E, KTOP = 1024, 4096, 1024, 8, 2
TEMP = 2.718281828459045
NCORES = 8

DK = D // P    # 8  k-subtiles for mm1
HK = H // P    # 32 k-subtiles for mm2
HT = H // P    # 32 h output tiles (mm1 M dim)
OT = O // P    # 8  o output tiles (mm2 M dim)
W2_KC = 4      # mm2 K chunks (each 8 subtiles = 1024)

# dtype config: (mm1 operand dt, mm2 operand dt == hT storage dt)
MM1_DT = mybir.dt.float32r
MM2_DT = mybir.dt.float32r
NPASS = 3

LAST_RESULTS = None  # BassKernelResults of the most recent device run (for test harness)


def _round_up(v, m):
    return -(-v // m) * m


def _split_subs(n):
    """Split n (multiple of 128) into free-dim blocks, each <=512 and >=256
    when possible (f32r runs 4x slower below 256 free dim)."""
    subs = []
    off = 0
    while n - off > 512:
        # avoid leaving a <256 tail
        take = 512 if (n - off - 512 == 0 or n - off - 512 >= 256) else 384
        subs.append((off, take))
        off += take
    if n - off:
        subs.append((off, n - off))
    return subs


def _pass_sizes(C):
    base = _round_up(-(-C // NPASS), P)
    sizes = []
    left = C
    while left > 0:
        s = min(base, left)
        sizes.append(s)
        left -= s
    return sizes


def _build_program(C):
    nc = bacc.Bacc("TRN2", target_bir_lowering=False, debug=False)

    xgT = nc.dram_tensor("xgT", (P, DK, C), MM1_DT, kind="ExternalInput")
    w1 = nc.dram_tensor("w1", (HT, P, DK, P), MM1_DT, kind="ExternalInput")
    w2 = nc.dram_tensor("w2", (OT, W2_KC, P, HK // W2_KC, P), MM2_DT, kind="ExternalInput")
    b1 = nc.dram_tensor("b1", (P, HT), mybir.dt.float32, kind="ExternalInput")
    yT = nc.dram_tensor("yT", (P, OT, C), mybir.dt.float32, kind="ExternalOutput")

    passes = _pass_sizes(C)

    with tile.TileContext(nc) as tc:
        with (
            tc.tile_pool(name="const", bufs=1) as const,
            tc.tile_pool(name="xg", bufs=1) as xg_pool,
            tc.tile_pool(name="ht", bufs=1) as ht_pool,
            tc.tile_pool(name="w1p", bufs=4) as w1_pool,
            tc.tile_pool(name="w2p", bufs=8) as w2_pool,
            tc.tile_pool(name="yst", bufs=3) as yst_pool,
            tc.tile_pool(name="psa", bufs=4, space="PSUM") as psa,
            tc.tile_pool(name="psb", bufs=3, space="PSUM") as psb,
        ):
            b1_sb = const.tile([P, HT], mybir.dt.float32)
            nc.sync.dma_start(b1_sb[:], b1.ap())

            off = 0
            for psize in passes:
                subs = _split_subs(psize)

                xg_t = xg_pool.tile([P, DK, passes[0]], MM1_DT, name="xg_t")
                nc.sync.dma_start(
                    xg_t[:, :, :psize], xgT.ap()[:, :, off : off + psize]
                )

                ht_t = ht_pool.tile([P, HK, passes[0]], MM2_DT, name="ht_t")

                # ---- mm1: hT = relu(W1^T @ xgT + b1) ----
                for ht in range(HT):
                    w1_t = w1_pool.tile([P, DK, P], MM1_DT, name="w1_t")
                    nc.sync.dma_start(w1_t[:], w1.ap()[ht])
                    for s0, sz in subs:
                        ps = psa.tile([P, 512], mybir.dt.float32, name="ps_a")
                        for k in range(DK):
                            nc.tensor.matmul(
                                ps[:, :sz],
                                w1_t[:, k, :],
                                xg_t[:, k, s0 : s0 + sz],
                                start=(k == 0),
                                stop=(k == DK - 1),
                            )
                        nc.scalar.activation(
                            ht_t[:, ht, s0 : s0 + sz],
                            ps[:, :sz],
                            mybir.ActivationFunctionType.Relu,
                            bias=b1_sb[:, ht : ht + 1],
                        )

                # ---- mm2: yT = W2^T @ hT ----
                for ot in range(OT):
                    w2_ts = []
                    for kc in range(W2_KC):
                        w2_t = w2_pool.tile([P, HK // W2_KC, P], MM2_DT, name="w2_t")
                        nc.sync.dma_start(w2_t[:], w2.ap()[ot, kc])
                        w2_ts.append(w2_t)
                    for s0, sz in subs:
                        ps = psb.tile([P, 512], mybir.dt.float32, name="ps_b")
                        for kc in range(W2_KC):
                            for k in range(HK // W2_KC):
                                nc.tensor.matmul(
                                    ps[:, :sz],
                                    w2_ts[kc][:, k, :],
                                    ht_t[:, kc * (HK // W2_KC) + k, s0 : s0 + sz],
                                    start=(kc == 0 and k == 0),
                                    stop=(kc == W2_KC - 1 and k == HK // W2_KC - 1),
                                )
                        st = yst_pool.tile([P, 512], mybir.dt.float32, name="y_st")
                        nc.vector.tensor_copy(st[:, :sz], ps[:, :sz])
                        nc.sync.dma_start(
                            yT.ap()[:, ot, off + s0 : off + s0 + sz], st[:, :sz]
                        )

                off += psize

    nc.compile()
    return nc


def _host_gate(x, Wg, bg):
    """Replicates reference gating in fp32: softmax(scores/T) -> top-2 -> renorm."""
    scores = (x @ Wg + bg) / np.float32(TEMP)
    m = scores.max(axis=-1, keepdims=True)
    un = np.exp(scores - m)
    probs = un / un.sum(-1, keepdims=True)
    order = np.argsort(-probs, axis=1, kind="stable")[:, :KTOP]
    vals = np.take_along_axis(probs, order, axis=1)
    w = np.zeros_like(probs)
    np.put_along_axis(w, order, vals, axis=1)
    w = w / (w.sum(-1, keepdims=True) + np.float32(1e-8))
    return w


def kernel(x, Wg, bg, W1, b1, W2, b2):
    global LAST_RESULTS
    x = np.ascontiguousarray(np.asarray(x, dtype=np.float32))
    Wg = np.asarray(Wg, dtype=np.float32)
    bg = np.asarray(bg, dtype=np.float32)
    W1 = np.asarray(W1, dtype=np.float32)
    b1 = np.asarray(b1, dtype=np.float32)
    W2 = np.asarray(W2, dtype=np.float32)
    b2 = np.asarray(b2, dtype=np.float32)
    N = x.shape[0]

    w = _host_gate(x, Wg, bg)  # [N, E] sparse renormalized top-2 weights

    idxs, counts = [], []
    for e in range(E):
        idx = np.nonzero(w[:, e])[0]
        idxs.append(idx)
        counts.append(len(idx))
    C = max(P, _round_up(max(counts), P))

    in_maps = []
    for e in range(E):
        idx = idxs[e]
        pad = np.zeros(C - len(idx), dtype=idx.dtype)
        idx_p = np.concatenate([idx, pad])
        xg = x[idx_p]  # [C, D]
        # xgT partition-major: [p, kd, c] = xg[c, kd*128 + p]
        xgT = np.ascontiguousarray(xg.T.reshape(DK, P, C).transpose(1, 0, 2))
        # w1 tiles: [ht, p, k, m] = W1[k*128+p, ht*128+m]
        w1_pm = np.ascontiguousarray(
            W1[e].reshape(DK, P, HT, P).transpose(2, 1, 0, 3)
        )
        # w2 tiles: [ot, kc, p, k, m] = W2[(kc*8+k)*128+p, ot*128+m]
        w2_pm = np.ascontiguousarray(
            W2[e]
            .reshape(W2_KC, HK // W2_KC, P, OT, P)
            .transpose(3, 0, 2, 1, 4)
        )
        b1_pm = np.ascontiguousarray(b1[e].reshape(HT, P).T)
        in_maps.append({"xgT": xgT, "w1": w1_pm, "w2": w2_pm, "b1": b1_pm})

    nc = _build_program(C)
    res = run_bass_kernel_spmd(nc, in_maps, core_ids=list(range(NCORES)))
    LAST_RESULTS = res

    out = np.zeros((N, O), dtype=np.float32)
    for e in range(E):
        c = counts[e]
        yT = res.results[e]["yT"]  # [P, OT, C]
        y = yT.transpose(1, 0, 2).reshape(O, C)[:, :c].T  # [c, O]
        out[idxs[e]] += w[idxs[e], e][:, None] * (y + b2[e])
    return out
